# revision 8
# baseline (speedup 1.0000x reference)
"""Bass/Tile kernel for the sharded NT-Xent contrastive loss (v2, fp8).

Per-core computation (core c of 8), B=8192 D=512 M=1024:
  inputs (host pre-transposed + cast to bf16):
    vt [512, 1024] bf16 : v[c*M:(c+1)*M, :].T  (this core's v-shard)
    ut [512, 8192] bf16 : u.T                   (full u)
    us [512, 1024] bf16 : u[c*M:(c+1)*M, :].T  (u-shard cols, for diagonal)
  output:
    loss [1024] f32 : loss rows c*M:(c+1)*M

  math (TEMP=0.5 so the exp argument is 2*cos):
    invu[j] = rsqrt(||u_j||^2) ; invv[i] = rsqrt(||v_i||^2)   (rsqrt = exp(-0.5*ln))
    un32 = u * invu * 32  cast fp8          vb = v cast fp8 (unnormalized)
    S[i,j] = sum_d vb[d,i]*un32[d,j]        (PE fp8 DoubleRow, psum f32)
    den[i] = sum_j exp((2*invv[i]/32) * S[i,j])   (ACT exp, accum_out)
    dot[i] = v_i . u_i (bf16 products, f32 colsum)   s2[i] = 2*invv[i]*invus[i]
    loss[i] = ln(exp(s2*dot) + den) - s2*dot

Pipeline: u is processed in 4 chunks of 2048 columns; chunk ch+1's
load/square/colsum/rsqrt/normalize overlaps chunk ch's matmul+exp main
loop. All activations use only Exp/Ln, and the activation-table map is
patched so both live in the single `natural_log_exp_and_others` set —
one ACT_TABLE_LOAD for the whole kernel.
"""

from contextlib import ExitStack

import concourse.bass as bass
import concourse.tile as tile
from concourse import bacc, mybir

F32 = mybir.dt.float32
BF16 = mybir.dt.bfloat16
FP8 = mybir.dt.float8e4
MULT = mybir.AluOpType.mult
ADD = mybir.AluOpType.add
SUB = mybir.AluOpType.subtract
AF = mybir.ActivationFunctionType
DR = mybir.MatmulPerfMode.DoubleRow

B = 8192
D = 512
NCORES = 8
M = B // NCORES   # 1024
KT = D // 128     # 4 d-tiles
NPAIR = KT // 2   # 2 DoubleRow d-tile pairs
NIT = M // 128    # 8 i-tiles
JC = 2048         # u-chunk columns
NCH = B // JC     # 4 chunks
USCALE = 32.0     # un is scaled by 32 before the fp8 cast


def _pin_act_tables():
    """Strip Exp/Ln from every activation-table set except
    natural_log_exp_and_others, so bass's table-load inserter is forced
    onto the one set that contains both (no mid-kernel table swaps)."""
    import functools

    import concourse.bacc as bacc_mod
    import concourse.hw_specs as hw_specs

    orig = hw_specs.get_activation_tables.__wrapped__

    @functools.cache
    def patched(arch):
        out = {}
        for name, fns in orig(arch).items():
            if name == "natural_log_exp_and_others":
                out[name] = fns
            else:
                out[name] = fns - {AF.Exp, AF.Ln}
        return out

    hw_specs.get_activation_tables = patched
    bacc_mod.get_activation_tables = patched


def _rsqrt_act(nc, pool, ss, cols, tag, scale=1.0):
    """rsqrt(ss*scale) = exp(-0.5*ln(ss*scale)) on ACT (Exp/Ln only)."""
    ln_t = pool.tile([128, cols], F32, tag=f"{tag}_ln")
    nc.scalar.activation(ln_t[:], ss[:], AF.Ln, scale=scale)
    inv = pool.tile([128, cols], F32, tag=f"{tag}_iv")
    nc.scalar.activation(inv[:], ln_t[:], AF.Exp, scale=-0.5)
    return inv


def build_nc():
    _pin_act_tables()
    nc = bacc.Bacc("TRN2", target_bir_lowering=False, debug=False,
                   num_devices=NCORES)

    vt = nc.dram_tensor("vt", [D, M], BF16, kind="ExternalInput")
    ut = nc.dram_tensor("ut", [D, B], BF16, kind="ExternalInput")
    us = nc.dram_tensor("us", [D, M], BF16, kind="ExternalInput")
    loss = nc.dram_tensor("loss", [M], F32, kind="ExternalOutput")
    bounce = [nc.dram_tensor(f"bounce{i}", [M], F32) for i in range(3)]

    with tile.TileContext(nc) as tc, ExitStack() as ctx:
        consts = ctx.enter_context(tc.tile_pool(name="consts", bufs=1))
        upool = ctx.enter_context(tc.tile_pool(name="upool", bufs=1))
        keep = ctx.enter_context(tc.tile_pool(name="keep", bufs=1))

        ones8 = consts.tile([128, 2, 128], FP8)
        nc.vector.memset(ones8[:], 1.0)
        ones_bf = consts.tile([128, 128], BF16)
        nc.vector.memset(ones_bf[:], 1.0)

        # persistent state
        scl = keep.tile([128, NIT], F32)      # 2*invv/32  (exp scale)
        s2 = keep.tile([128, NIT], F32)       # 2*invv*invus
        dot_t = keep.tile([128, NIT], F32)    # v_i . u_i (f32)
        den = keep.tile([128, NIT], F32)      # sum_j exp(...)
        nc.vector.memset(den[:], 0.0)
        # fp8 DoubleRow operands: [128, 2, cols]; pair p holds d-tiles 2p,2p+1
        unf = [upool.tile([128, 2, B], FP8, tag=f"unf{p}", name=f"unf{p}")
               for p in range(NPAIR)]
        vbf = [keep.tile([128, 2, M], FP8, tag=f"vbf{p}", name=f"vbf{p}")
               for p in range(NPAIR)]

        # =================== v prologue ===================
        with tc.tile_pool(name="vstage", bufs=1) as vst, \
             tc.tile_pool(name="vsq", bufs=1) as vsq, \
             tc.tile_pool(name="vflat", bufs=1) as flpool, \
             tc.tile_pool(name="vsmall", bufs=1) as small, \
             tc.tile_pool(name="vpsum", bufs=1, space="PSUM") as vps:

            vstage, usstage = [], []
            for dt in range(KT):
                sv = vst.tile([128, M], BF16, tag=f"vs{dt}")
                nc.sync.dma_start(sv[:], vt.ap()[dt * 128:(dt + 1) * 128, :])
                vstage.append(sv)
                su = vst.tile([128, M], BF16, tag=f"us{dt}")
                nc.sync.dma_start(su[:], us.ap()[dt * 128:(dt + 1) * 128, :])
                usstage.append(su)

            # fp8 squares (interleaved pair layout) + bf16 v*u products
            sqv = [vsq.tile([128, 2, M], FP8, tag=f"sqv{p}", name=f"sqv{p}")
                   for p in range(NPAIR)]
            squs = [vsq.tile([128, 2, M], FP8, tag=f"squs{p}", name=f"squs{p}")
                    for p in range(NPAIR)]
            prods = []
            for dt in range(KT):
                p, t = divmod(dt, 2)
                nc.vector.tensor_tensor(sqv[p][:, t, :], vstage[dt][:],
                                        vstage[dt][:], MULT)
                nc.vector.tensor_tensor(squs[p][:, t, :], usstage[dt][:],
                                        usstage[dt][:], MULT)
                pr = vsq.tile([128, M], BF16, tag=f"prod{dt}", name=f"prod{dt}")
                nc.vector.tensor_tensor(pr[:], vstage[dt][:], usstage[dt][:],
                                        MULT)
                prods.append(pr)
                # cast this core's v shard to fp8 (unnormalized)
                nc.vector.tensor_copy(vbf[p][:, t, :], vstage[dt][:])

            ps_v = vps.tile([128, M], F32, tag="psv")
            ps_us = vps.tile([128, M], F32, tag="psus")
            ps_dot = vps.tile([128, M], F32, tag="psdot")
            for jc in range(M // 512):
                sl = slice(jc * 512, (jc + 1) * 512)
                for p in range(NPAIR):
                    nc.tensor.matmul(ps_v[:, sl], lhsT=ones8[:],
                                     rhs=sqv[p][:, :, sl],
                                     start=(p == 0), stop=(p == NPAIR - 1),
                                     perf_mode=DR)
                for p in range(NPAIR):
                    nc.tensor.matmul(ps_us[:, sl], lhsT=ones8[:],
                                     rhs=squs[p][:, :, sl],
                                     start=(p == 0), stop=(p == NPAIR - 1),
                                     perf_mode=DR)
                for dt in range(KT):
                    nc.tensor.matmul(ps_dot[:, sl], lhsT=ones_bf[:],
                                     rhs=prods[dt][:, sl],
                                     start=(dt == 0), stop=(dt == KT - 1))

            # compact [1, M] rows -> [128, NIT] via DRAM bounce
            ssv_t = small.tile([128, NIT], F32, tag="ssv")
            ssus_t = small.tile([128, NIT], F32, tag="ssus")
            for k, (ps_acc, dst) in enumerate(
                    ((ps_v, ssv_t), (ps_us, ssus_t), (ps_dot, dot_t))):
                fl = flpool.tile([1, M], F32, tag=f"flat{k}")
                nc.scalar.copy(fl[:], ps_acc[0:1, :])
                nc.sync.dma_start(bounce[k].ap(), fl[:])
                nc.sync.dma_start(
                    dst[:], bounce[k].ap().rearrange("(t p) -> p t", p=128))

            invv_t = _rsqrt_act(nc, small, ssv_t, NIT, "rv")
            invus_t = _rsqrt_act(nc, small, ssus_t, NIT, "rs")
            nc.vector.tensor_scalar(scl[:], invv_t[:], 2.0 / USCALE, None, MULT)
            nc.vector.tensor_tensor(s2[:], invv_t[:], invus_t[:], MULT)
            nc.vector.tensor_scalar(s2[:], s2[:], 2.0, None, MULT)

        # =================== u chunks + main loop (pipelined) ===============
        # created after the v-prologue pools release their PSUM banks
        stpool = ctx.enter_context(tc.tile_pool(name="stage", bufs=2))
        sqpool = ctx.enter_context(tc.tile_pool(name="sq", bufs=2))
        nrmpool = ctx.enter_context(tc.tile_pool(name="nrm", bufs=2))
        expool = ctx.enter_context(tc.tile_pool(name="ex", bufs=2))
        dpool = ctx.enter_context(tc.tile_pool(name="dp", bufs=4))
        mps = ctx.enter_context(tc.tile_pool(name="mpsum", bufs=2, space="PSUM"))

        def u_chunk(ch):
            """load chunk, squares, colsum, rsqrt, normalize+cast to fp8."""
            c0 = ch * JC
            stages = []
            for dt in range(KT):
                st = stpool.tile([128, JC], BF16, tag=f"ust{dt}",
                                 name=f"ust{dt}_{ch}")
                nc.sync.dma_start(
                    st[:], ut.ap()[dt * 128:(dt + 1) * 128, c0:c0 + JC])
                stages.append(st)
            squ = [sqpool.tile([128, 2, JC], FP8, tag=f"squ{p}",
                               name=f"squ{p}_{ch}") for p in range(NPAIR)]
            for dt in range(KT):
                p, t = divmod(dt, 2)
                nc.vector.tensor_tensor(squ[p][:, t, :], stages[dt][:],
                                        stages[dt][:], MULT)
            ss_ps = mps.tile([128, JC], F32, tag="mm", name=f"ss{ch}")
            for jc in range(JC // 512):
                sl = slice(jc * 512, (jc + 1) * 512)
                for p in range(NPAIR):
                    nc.tensor.matmul(ss_ps[:, sl], lhsT=ones8[:],
                                     rhs=squ[p][:, :, sl],
                                     start=(p == 0), stop=(p == NPAIR - 1),
                                     perf_mode=DR)
            # invu*32 = rsqrt(ss/1024), replicated on all partitions
            ln_t = nrmpool.tile([128, JC], F32, tag="lnss", name=f"lnss{ch}")
            nc.scalar.activation(ln_t[:], ss_ps[:], AF.Ln,
                                 scale=1.0 / (USCALE * USCALE))
            inv_t = nrmpool.tile([128, JC], BF16, tag="invu", name=f"invu{ch}")
            nc.scalar.activation(inv_t[:], ln_t[:], AF.Exp, scale=-0.5)
            for dt in range(KT):
                p, t = divmod(dt, 2)
                nc.vector.tensor_tensor(unf[p][:, t, c0:c0 + JC],
                                        stages[dt][:], inv_t[:], MULT)

        def main_its(ch):
            """matmul + exp row-sums against u chunk ch, all 8 i-tiles."""
            c0 = ch * JC
            for it in range(NIT):
                ps = mps.tile([128, JC], F32, tag="mm", name=f"mm{ch}_{it}")
                for p in range(NPAIR):
                    lw = vbf[p][:, :, it * 128:(it + 1) * 128]
                    for jc in range(JC // 512):
                        nc.tensor.matmul(
                            ps[:, jc * 512:(jc + 1) * 512], lhsT=lw,
                            rhs=unf[p][:, :, c0 + jc * 512:c0 + (jc + 1) * 512],
                            start=(p == 0), stop=(p == NPAIR - 1),
                            perf_mode=DR)
                ex = expool.tile([128, JC], BF16, tag="ex")
                dp = dpool.tile([128, 1], F32, tag="dp")
                nc.scalar.activation(ex[:], ps[:], AF.Exp,
                                     scale=scl[:, it:it + 1], accum_out=dp[:])
                nc.vector.tensor_tensor(den[:, it:it + 1], den[:, it:it + 1],
                                        dp[:], ADD)

        u_chunk(0)
        for ch in range(1, NCH):
            u_chunk(ch)
            main_its(ch - 1)
        main_its(NCH - 1)

        # =================== epilogue ===================
        with tc.tile_pool(name="fin", bufs=1) as fin:
            t2 = fin.tile([128, NIT], F32, tag="t2")
            nc.vector.tensor_tensor(t2[:], s2[:], dot_t[:], MULT)
            numt = fin.tile([128, NIT], F32, tag="numt")
            nc.scalar.activation(numt[:], t2[:], AF.Exp)
            dtot = fin.tile([128, NIT], F32, tag="dtot")
            nc.vector.tensor_tensor(dtot[:], den[:], numt[:], ADD)
            lg = fin.tile([128, NIT], F32, tag="lg")
            nc.scalar.activation(lg[:], dtot[:], AF.Ln)
            lt = fin.tile([128, NIT], F32, tag="lt")
            nc.vector.tensor_tensor(lt[:], lg[:], t2[:], SUB)
            nc.sync.dma_start(
                loss.ap().rearrange("(t p) -> p t", p=128), lt[:])

    nc.compile()
    return nc


# ======================================================================
# Host-side entry point: full inputs in, full output out.
# ======================================================================
import numpy as np

_NC_CACHE = {}


def _get_nc():
    if "nc" not in _NC_CACHE:
        _NC_CACHE["nc"] = build_nc()
    return _NC_CACHE["nc"]


def make_in_maps(v: np.ndarray, u: np.ndarray):
    import ml_dtypes

    bf = ml_dtypes.bfloat16
    vT = np.ascontiguousarray(v.T.astype(bf))   # [D, B]
    uT = np.ascontiguousarray(u.T.astype(bf))   # [D, B]
    in_maps = []
    for c in range(NCORES):
        sl = slice(c * M, (c + 1) * M)
        in_maps.append({
            "vt": np.ascontiguousarray(vT[:, sl]),
            "ut": uT,
            "us": np.ascontiguousarray(uT[:, sl]),
        })
    return in_maps


def kernel(v: np.ndarray, u: np.ndarray) -> np.ndarray:
    from concourse.bass_utils import run_bass_kernel_spmd

    nc = _get_nc()
    v = np.asarray(v, dtype=np.float32)
    u = np.asarray(u, dtype=np.float32)
    in_maps = make_in_maps(v, u)
    res = run_bass_kernel_spmd(nc, in_maps, core_ids=list(range(NCORES)))
    return np.concatenate([res.results[c]["loss"] for c in range(NCORES)])
